# revision 12
# baseline (speedup 1.0000x reference)
"""Multi-head attention (B=2, N=2048, C=1024, H=16, D=64) on 8 TRN2 NeuronCores.

Sharding: core c = (batch b = c//4) x (head-group g = c%4 -> heads 4g..4g+3).
Data parallel on B, tensor parallel on heads.  After each head's softmax
normalization, the [64, cw] head-output is AllGathered (fp16) within the
4-core batch group; each core then runs the full-K (1024) out-projection
for its 256-channel output slice locally -- no reduce, no post-projection
collective, and the gathers overlap the attention pipeline.

Everything on device stays transposed ([channel, position]); the host
pre-transposes inputs and post-transposes the output.
"""

import numpy as np

import concourse.bacc as bacc
import concourse.tile as tile
import concourse.mybir as mybir
from concourse.bass_utils import run_bass_kernel_spmd

B, N, C, H = 2, 2048, 1024, 16
D = C // H          # 64
HL = H // 4         # 4 heads per core
CL = HL * D         # 256 local channels
N_CORES = 8
GROUPS = [[0, 1, 2, 3], [4, 5, 6, 7]]

F32 = mybir.dt.float32
BF16 = mybir.dt.float16
BF = np.float16

KC = C // 128       # 8  K-chunks of the input channel dim
NI = N // 512       # 4  512-wide i-chunks
NJ = N // 128       # 16 128-row j-chunks


def build_kernel(n_cores=N_CORES, groups=GROUPS):
    group_size = len(groups[0])
    out_rows = C // group_size          # 256 output channels per core

    nc = bacc.Bacc("TRN2", target_bir_lowering=False, debug=False,
                   num_devices=n_cores)

    xT = nc.declare_dram_parameter("xT", [C, N], BF16, isOutput=False)
    cos2 = nc.declare_dram_parameter("cos2", [128, N], BF16, isOutput=False)
    sin2s = nc.declare_dram_parameter("sin2s", [128, N], BF16, isOutput=False)
    wqkT = nc.declare_dram_parameter("wqkT", [C, 2 * CL], BF16, isOutput=False)
    bqk = nc.declare_dram_parameter("bqk", [2 * CL, 1], F32, isOutput=False)
    wvT = nc.declare_dram_parameter("wvT", [C, CL], BF16, isOutput=False)
    # out-projection weights, rows ordered (head_local, rank, d) to match
    # the AllGather concat order; cols = this core's 256 output channels
    wprojT = nc.declare_dram_parameter("wprojT", [C, out_rows], BF16,
                                       isOutput=False)
    beff = nc.declare_dram_parameter("beff", [out_rows, 1], F32, isOutput=False)
    out = nc.declare_dram_parameter("out", [out_rows, N], F32, isOutput=True)

    with tile.TileContext(nc) as tc:
        with tc.tile_pool(name="dram", bufs=1, space="DRAM") as dram, \
             tc.tile_pool(name="sbuf", bufs=1) as sb, \
             tc.tile_pool(name="psum", bufs=1, space="PSUM") as ps:

            # tile for clock-warming matmuls (see _warm_pe)
            warm = sb.tile([128, 128], BF16, name="warm", tag="warm")
            nc.vector.memset(warm[:], 0.001)

            def _warm_pe(tag, n):
                # short matmuls alternating two PSUM tiles: keeps the PE's
                # activity monitor busy so the clock gate stays at full rate
                wps = [ps.tile([128, 64], F32, name=f"warmp{tag}_{a}",
                               tag="sc", bufs=2) for a in range(2)]
                for r in range(n):
                    nc.tensor.matmul(wps[r % 2][:], warm[:], warm[:, :64],
                                     start=True, stop=True)

            # run a warm burst during the input-DMA dead window so the qk
            # projection starts with the clock gate already released
            _warm_pe("s", 24)

            # ---- load inputs (wqk/xb interleaved so the qk matmuls can start
            # before the full x transfer lands) ----
            # both HWDGE queues (sync + scalar) share the bulk input load
            xb, wqk_sb = [], []
            for kc in range(KC):
                t = sb.tile([128, 2 * CL], BF16, name=f"wqk{kc}", tag=f"wqk{kc}")
                eng = nc.scalar if kc % 2 == 0 else nc.sync
                eng.dma_start(t[:], wqkT.ap()[128 * kc:128 * (kc + 1), :])
                wqk_sb.append(t)
                t = sb.tile([128, N], BF16, name=f"xb{kc}", tag=f"xb{kc}")
                eng = nc.sync if kc % 2 == 0 else nc.scalar
                eng.dma_start(t[:], xT.ap()[128 * kc:128 * (kc + 1), :])
                xb.append(t)
            wv_sb = []
            for kc in range(KC):
                t = sb.tile([128, CL], BF16, name=f"wv{kc}", tag=f"wv{kc}")
                nc.sync.dma_start(t[:], wvT.ap()[128 * kc:128 * (kc + 1), :])
                wv_sb.append(t)
            cos_sb = sb.tile([128, N], BF16, name="cos_sb", tag="cos_sb")
            nc.sync.dma_start(cos_sb[:], cos2.ap())
            sin_sb = sb.tile([128, N], BF16, name="sin_sb", tag="sin_sb")
            nc.scalar.dma_start(sin_sb[:], sin2s.ap())
            bqk_sb = []
            for m in range(4):
                t = sb.tile([128, 1], F32, name=f"bqk{m}", tag=f"bqk{m}")
                nc.sync.dma_start(t[:], bqk.ap()[128 * m:128 * (m + 1), :])
                bqk_sb.append(t)
            wproj_sb = []
            for kc in range(KC):
                t = sb.tile([128, out_rows], BF16, name=f"wproj{kc}",
                            tag=f"wproj{kc}")
                nc.sync.dma_start(t[:], wprojT.ap()[128 * kc:128 * (kc + 1), :])
                wproj_sb.append(t)
            beff_sb = []
            for m in range(out_rows // 128):
                t = sb.tile([128, 1], F32, name=f"beff{m}", tag=f"beff{m}")
                nc.sync.dma_start(t[:], beff.ap()[128 * m:128 * (m + 1), :])
                beff_sb.append(t)

            # ---- qk projection + RoPE ----
            # chunk m rows: m=0:[q_h0,q_h1] m=1:[q_h2,q_h3] m=2:[k_h0,k_h1] m=3:[k_h2,k_h3]
            # so q and k of head h sit at the same partition offset 64*(h%2).
            # k of each head lands in its own zero-padded [128, N] tile so the
            # scores matmul can contract over K=128 (16-bit matmuls run at
            # half rate for K=64 -- zero rows buy back the full rate).
            q_r = []      # 2 tiles: [q_h0,q_h1], [q_h2,q_h3]
            k_t = []      # 4 tiles: k_h at rows 64*(h%2), zeros elsewhere
            for h in range(4):
                kt = sb.tile([128, N], BF16, name=f"ktile{h}", tag=f"ktile{h}")
                z = slice(0, 64) if h % 2 == 1 else slice(64, 128)
                nc.vector.memset(kt[z, :], 0.0)
                k_t.append(kt)
            swap_mask = [i ^ 1 for i in range(32)]
            # kc-outer accumulation so the first matmul only needs the first
            # x/w chunk off DMA; 2 PSUM tiles hold the 4 m-accumulators
            qks_t = [sb.tile([128, N], BF16, name=f"qks{m}", tag=f"qks{m}")
                     for m in range(4)]
            for n in range(NI):
                accs = [ps.tile([128, 1024], F32, name=f"qacc{n}_{a}",
                                tag="sc", bufs=2) for a in range(2)]
                for kc in range(KC):
                    for m in range(4):
                        nc.tensor.matmul(
                            accs[m // 2][:, 512 * (m % 2):512 * (m % 2 + 1)],
                            wqk_sb[kc][:, 128 * m:128 * (m + 1)],
                            xb[kc][:, 512 * n:512 * (n + 1)],
                            start=(kc == 0), stop=(kc == KC - 1))
                for m in range(4):
                    nc.scalar.activation(
                        qks_t[m][:, 512 * n:512 * (n + 1)],
                        accs[m // 2][:, 512 * (m % 2):512 * (m % 2 + 1)],
                        mybir.ActivationFunctionType.Identity,
                        bias=bqk_sb[m][:])
            for m in range(4):
                qks = qks_t[m]
                # RoPE: qk' = qks*cos2 + shift(qks)*sin2s
                # (pair-swap of adjacent partitions via DVE stream shuffle)
                shf = sb.tile([128, N], BF16, name=f"shf{m}", tag="shf", bufs=2)
                nc.vector.stream_shuffle(shf[:], qks[:], swap_mask)
                t2 = sb.tile([128, N], BF16, name=f"ropetmp{m}", tag="ropetmp", bufs=2)
                nc.vector.tensor_mul(t2[:], shf[:], sin_sb[:])
                if m < 2:
                    qkr = sb.tile([128, N], BF16, name=f"qkr{m}", tag=f"qkr{m}")
                    nc.vector.tensor_mul(qkr[:], qks[:], cos_sb[:])
                    nc.vector.tensor_add(qkr[:], qkr[:], t2[:])
                    q_r.append(qkr)
                else:
                    t1 = sb.tile([128, N], BF16, name=f"ropetc{m}", tag="ropetc",
                                 bufs=2)
                    nc.vector.tensor_mul(t1[:], qks[:], cos_sb[:])
                    h0, h1 = 2 * (m - 2), 2 * (m - 2) + 1
                    nc.vector.tensor_add(k_t[h0][0:64, :], t1[0:64, :],
                                         t2[0:64, :])
                    nc.vector.tensor_add(k_t[h1][64:128, :], t1[64:128, :],
                                         t2[64:128, :])

            # ---- v projection (natural [j, ch] layout, ones col appended per head) ----
            # j-chunks processed in pairs with the matmul stream alternating
            # between the two accumulators: back-to-back matmuls into the
            # same PSUM address serialize (~+330ns each), alternating ones
            # pipeline
            vaug = [None] * NJ
            for jp in range(NJ // 2):
                jcs = (2 * jp, 2 * jp + 1)
                pvs = [ps.tile([128, CL], F32, name=f"pv{jc}", tag="sc",
                               bufs=2) for jc in jcs]
                for kc in range(KC):
                    for a, jc in enumerate(jcs):
                        nc.tensor.matmul(
                            pvs[a][:],
                            xb[kc][:, 128 * jc:128 * (jc + 1)],
                            wv_sb[kc][:],
                            start=(kc == 0), stop=(kc == KC - 1))
                for a, jc in enumerate(jcs):
                    va = sb.tile([128, HL * (D + 1)], BF16, name=f"vaug{jc}",
                                 tag=f"vaug{jc}")
                    nc.vector.memset(va[:, D::D + 1], 1.0)
                    nc.scalar.activation(
                        va.rearrange("p (h e) -> p h e", e=D + 1)[:, :, 0:D],
                        pvs[a].rearrange("p (h e) -> p h e", e=D)[:, :, :],
                        mybir.ActivationFunctionType.Copy)
                    vaug[jc] = va

            # warm up the collective path during the preamble so the first
            # real gather doesn't absorb the ~20us first-collective cost
            agw_in = dram.tile([64, 8], BF16, name="agw_in", tag="agw_in")
            agw_out = dram.tile([64 * group_size, 8], BF16, name="agw_out",
                                tag="agw_out")
            agw_sb = sb.tile([64, 8], BF16, name="agw_sb", tag="agw_sb")
            nc.vector.memset(agw_sb[:], 0.0)
            nc.sync.dma_start(agw_in[:], agw_sb[:])
            nc.gpsimd.collective_compute(
                "AllGather", mybir.AluOpType.bypass, replica_groups=groups,
                ins=[agw_in[:]], outs=[agw_out[:]])

            # per-partition bias AP used to shift scores before fp16 exp
            eshift = sb.tile([128, 1], F32, name="eshift", tag="eshift")
            nc.vector.memset(eshift[:], -16.0)
            # K=1 ones row used to broadcast denominators across partitions
            ones64 = sb.tile([1, 64], BF16, name="ones64", tag="ones64")
            nc.vector.memset(ones64[:], 1.0)

            # ---- attention, per i-chunk; per-head AllGather of the
            # normalized output; out-projection jobs for chunk ih-1
            # interleaved into chunk ih's head pipeline ----
            chunks = [(0, 1024), (1024, 1024)]
            n_chunks = len(chunks)

            # DRAM staging for the head-output gathers
            ag_in = [[dram.tile([64, cw], BF16, name=f"agin{ih}_{hl}",
                                tag=f"agin{ih}_{hl}")
                      for hl in range(4)] for ih, (i0, cw) in enumerate(chunks)]
            ag_out = [[dram.tile([64 * group_size, cw], BF16,
                                 name=f"agout{ih}_{hl}", tag=f"agout{ih}_{hl}")
                       for hl in range(4)] for ih, (i0, cw) in enumerate(chunks)]

            # AGs fired so far, in order; readbacks are emitted two head-slots
            # after the AG fires so a pending readback on the gpsimd queue
            # never sits between two AG triggers (that would serialize the
            # collective stream: AG k+1 couldn't trigger until AG k finished)
            ag_fired = []
            rb_state = {"done": 0}

            def drain_readbacks(limit=1):
                while len(ag_fired) - rb_state["done"] > limit:
                    fih, fhl, fcw = ag_fired[rb_state["done"]]
                    emit_readback(fih, fhl, fcw)
                    rb_state["done"] += 1

            def finalize_head(ih, hl, oacc, cw):
                # normalize: o[:, i] / den[i].  Broadcast den across
                # partitions with a K=1 matmul (no DMA: DMA triggers can
                # block an engine queue while collective SDMA is in flight),
                # then reciprocal+mul on 64 partitions.
                den = sb.tile([1, cw], BF16, name=f"den{ih}_{hl}",
                              tag="den", bufs=2)
                nc.vector.tensor_copy(den[:], oacc[64:65, :])
                rb = ps.tile([64, cw], F32, name=f"rb{ih}_{hl}",
                             tag="oacc", bufs=2)
                for q in range(cw // 512):
                    nc.tensor.matmul(rb[:, 512 * q:512 * (q + 1)],
                                     ones64[:],
                                     den[:, 512 * q:512 * (q + 1)],
                                     start=True, stop=True)
                rr = sb.tile([64, cw], F32, name=f"rr{ih}_{hl}", tag="rr",
                             bufs=2)
                nc.vector.reciprocal_approx_fast(rr[:], rb[:])
                oh = sb.tile([64, cw], BF16, name=f"oh{ih}_{hl}", tag="oh",
                             bufs=2)
                nc.vector.tensor_mul(oh[:], oacc[0:64, :], rr[:])
                # stage to DRAM and gather the 4 cores' [64, cw] head outputs
                nc.sync.dma_start(ag_in[ih][hl][:], oh[:])
                nc.gpsimd.collective_compute(
                    "AllGather", mybir.AluOpType.bypass,
                    replica_groups=groups,
                    ins=[ag_in[ih][hl][:]],
                    outs=[ag_out[ih][hl][:]])
                ag_fired.append((ih, hl, cw))

            # gathered o readback tiles, kc = hl*2 + half (row order matches
            # wprojT's (head_local, rank, d) ordering)
            o_rb = [[None] * KC for _ in range(n_chunks)]

            def emit_readback(ih, hl, cw):
                for half in range(2):
                    t = sb.tile([128, cw], BF16, name=f"orb{ih}_{hl}_{half}",
                                tag="orb", bufs=12)
                    nc.gpsimd.dma_start(
                        t[:], ag_out[ih][hl][128 * half:128 * (half + 1), :])
                    o_rb[ih][2 * hl + half] = t

            def emit_proj_job(ih, mc, n2, i0, cw):
                # one out-projection quadrant: out rows [128*mc, 128*(mc+1)),
                # q cols [i0 + 512*n2, ...), contracting all 8 gathered kc
                # tiles of chunk ih.  PSUM via the sc rotation (1 bank).
                pp = ps.tile([128, 512], F32, name=f"pp{ih}_{mc}_{n2}",
                             tag="sc", bufs=2)
                isl = slice(512 * n2, 512 * (n2 + 1))
                for kc in range(KC):
                    nc.tensor.matmul(
                        pp[:],
                        wproj_sb[kc][:, 128 * mc:128 * (mc + 1)],
                        o_rb[ih][kc][:, isl],
                        start=(kc == 0), stop=(kc == KC - 1))
                fo = sb.tile([128, 512], F32, name=f"fo{ih}_{mc}_{n2}",
                             tag="fo", bufs=4)
                nc.vector.tensor_scalar_add(fo[:], pp[:], beff_sb[mc][:])
                nc.sync.dma_start(
                    out.ap()[128 * mc:128 * (mc + 1),
                             i0 + 512 * n2:i0 + 512 * (n2 + 1)], fo[:])

            for ih, (i0, cw) in enumerate(chunks):
                ns = cw // 512
                if ih > 0:
                    _warm_pe(f"c{ih}", 12)
                # out-projection jobs of the previous chunk, spread across
                # this chunk's head boundaries (job list filled below)
                prev_jobs = []
                if ih > 0:
                    pi0 = chunks[ih - 1][0]
                    pcw = chunks[ih - 1][1]
                    prev_jobs = [(ih - 1, mc, n2, pi0, pcw)
                                 for mc in range(out_rows // 128)
                                 for n2 in range(pcw // 512)]
                pending = None
                for hl in range(4):
                    qT = q_r[hl // 2]
                    kT = k_t[hl]
                    oacc = ps.tile([65, cw], F32, name=f"oacc{ih}_{hl}",
                                   tag="oacc", bufs=2)
                    exs = []

                    def emit_o(jc, oacc=oacc, exs=exs, hl=hl):
                        for q in range(ns):
                            nc.tensor.matmul(
                                oacc[:, 512 * q:512 * (q + 1)],
                                vaug[jc][:, (D + 1) * hl:(D + 1) * (hl + 1)],
                                exs[jc][:, 512 * q:512 * (q + 1)],
                                start=(jc == 0), stop=(jc == NJ - 1))

                    for jc in range(NJ):
                        sc = ps.tile([128, cw], F32, name=f"sc{ih}_{hl}_{jc}",
                                     tag="sc", bufs=2)
                        for q in range(ns):
                            nc.tensor.matmul(
                                sc[:, 512 * q:512 * (q + 1)],
                                kT[:, 128 * jc:128 * (jc + 1)],
                                qT[:, i0 + 512 * q:i0 + 512 * (q + 1)],
                                start=True, stop=True)
                        ex = sb.tile([128, cw], BF16, name=f"ex{ih}_{hl}_{jc}",
                                     tag="ex", bufs=3)
                        # bias shifts all scores so fp16 exp can't overflow
                        # (softmax is shift-invariant, cancels in num/den)
                        nc.scalar.activation(ex[:], sc[:],
                                             mybir.ActivationFunctionType.Exp,
                                             scale=float(1.0 / np.sqrt(D)),
                                             bias=eshift[:])
                        exs.append(ex)
                        # software pipeline: o-matmuls lag one j-chunk, and
                        # the previous head's normalization chain is deferred
                        # into this head's pipeline so the exp stream never
                        # pauses at head boundaries
                        if jc == 1 and pending is not None:
                            finalize_head(*pending)
                            pending = None
                        if jc == 2 and hl >= 2 and prev_jobs:
                            # previous chunk's projection quadrants (PE
                            # slack); not before head 2: the last gather of
                            # the previous chunk lands mid-head-1
                            emit_proj_job(*prev_jobs.pop(0))
                            emit_proj_job(*prev_jobs.pop(0))
                        if jc >= 1:
                            emit_o(jc - 1)
                    emit_o(NJ - 1)
                    pending = (ih, hl, oacc, cw)
                    # emit readbacks for gathers fired two head-slots ago
                    drain_readbacks(limit=1)
                # keep the PE clock-gate warm through the last head's
                # normalization chain so the projection starts at full rate
                _warm_pe(f"p{ih}", 12)
                finalize_head(*pending)
                pending = None

            # drain remaining readbacks, then the last chunk's
            # out-projection: exposed tail, starts when the final gather lands
            drain_readbacks(limit=0)
            li0, lcw = chunks[-1]
            for mc in range(out_rows // 128):
                for n2 in range(lcw // 512):
                    emit_proj_job(n_chunks - 1, mc, n2, li0, lcw)

    nc.compile()
    return nc


def shard_inputs(x, rope, w_qkv, b_qkv, w_proj, b_proj,
                 n_cores=N_CORES, group_size=4):
    """Per-core input maps. Host-side transposes/casts are part of sharding."""
    out_rows = C // group_size
    # fold the v-bias through the projection into an effective output bias
    b_v = b_qkv[2 * C:3 * C]
    b_eff = (b_proj + b_v @ w_proj.T).astype(np.float32)   # [C]

    in_maps = []
    for c in range(n_cores):
        b = (c // group_size) % B
        g = c % group_size
        heads = range(HL * g, HL * g + HL)

        xTb = np.ascontiguousarray(x[b].T).astype(BF)            # [C, N]

        cosT = rope[b].T[:D, :]                                   # [64, N]
        sinT = rope[b].T[D:, :]
        cos2 = np.vstack([cosT, cosT]).astype(BF)                 # [128, N]
        sgn = np.where(np.arange(128) % 2 == 0, -1.0, 1.0)[:, None]
        sin2s = (np.vstack([sinT, sinT]) * sgn).astype(BF)        # [128, N]

        # qk weight rows ordered [q_h0..q_h3, k_h0..k_h3]
        qk_rows = []
        bqk_rows = []
        for h in heads:
            qk_rows.append(w_qkv[D * h:D * (h + 1), :])           # q rows
            bqk_rows.append(b_qkv[D * h:D * (h + 1)])
        for h in heads:
            qk_rows.append(w_qkv[C + D * h:C + D * (h + 1), :])   # k rows
            bqk_rows.append(b_qkv[C + D * h:C + D * (h + 1)])
        wqk = np.vstack(qk_rows)                                  # [512, C]
        wqkT = np.ascontiguousarray(wqk.T).astype(BF)             # [C, 512]
        bqk_v = np.concatenate(bqk_rows).astype(np.float32)[:, None]

        h0 = HL * g
        wv = w_qkv[2 * C + D * h0:2 * C + D * h0 + CL, :]          # [256, C]
        wvT = np.ascontiguousarray(wv.T).astype(BF)                # [C, 256]

        # out-projection: this core owns output channels
        # [out_rows*g, out_rows*(g+1)).  The contraction rows are ordered
        # (head_local hl, rank r, d) to match the per-head AllGather layout
        # (gather hl concatenates rank blocks of 64 rows).
        osl = slice(out_rows * g, out_rows * (g + 1))
        wp_rows = []
        for hl in range(HL):
            for r in range(group_size):
                gh = HL * r + hl                   # global head of (r, hl)
                wp_rows.append(w_proj[osl, D * gh:D * (gh + 1)].T)  # [64,256]
        wprojT = np.ascontiguousarray(np.vstack(wp_rows)).astype(BF)  # [C,256]

        beff_shard = b_eff[osl].astype(np.float32)[:, None]

        in_maps.append({
            "xT": xTb, "cos2": cos2, "sin2s": sin2s,
            "wqkT": wqkT, "bqk": bqk_v, "wvT": wvT,
            "wprojT": wprojT, "beff": beff_shard,
        })
    return in_maps


def assemble(results, n_cores=N_CORES, group_size=4):
    out_rows = C // group_size
    out = np.empty((B, N, C), dtype=np.float32)
    for c in range(n_cores):
        b = (c // group_size) % B
        g = c % group_size
        outT_shard = results[c]["out"]                 # [out_rows, N] f32
        out[b, :, out_rows * g:out_rows * (g + 1)] = outT_shard.T
    return out


_NC_CACHE = {}


def _get_nc():
    if "nc" not in _NC_CACHE:
        _NC_CACHE["nc"] = build_kernel()
    return _NC_CACHE["nc"]


def _run(inputs, trace=False, tmpdir=None):
    nc = _get_nc()
    inputs = {k: np.asarray(v) for k, v in inputs.items()}
    in_maps = shard_inputs(**inputs)
    res = run_bass_kernel_spmd(nc, in_maps, core_ids=list(range(N_CORES)),
                               trace=trace, tmpdir=tmpdir)
    return assemble(res.results), res


def kernel(**inputs):
    out, _ = _run(inputs)
    return out


# revision 14
# speedup vs baseline: 1.0944x; 1.0944x over previous
"""Multi-head attention (B=2, N=2048, C=1024, H=16, D=64) on 8 TRN2 NeuronCores.

Sharding: core c = (batch b = c//4) x (head-group g = c%4 -> heads 4g..4g+3).
Data parallel on B, tensor parallel on heads.  After each head's softmax
normalization, the [64, cw] head-output is AllGathered (fp16) within the
4-core batch group; each core then runs the full-K (1024) out-projection
for its 256-channel output slice locally -- no reduce, no post-projection
collective, and the gathers overlap the attention pipeline.

The projections are software-pipelined into the attention stream: only the
k-projection (m2) and the first half of the q-projection (m0 @ n0,n1) run
before head 0 starts; the remaining qk projection chunks, the whole
v-projection and the previous chunk's out-projection quadrants are folded
into the attention pipeline's PE slack (the inner loop is ACT/exp-bound).

Everything on device stays transposed ([channel, position]); the host
pre-transposes inputs and post-transposes the output.
"""

import numpy as np

import concourse.bacc as bacc
import concourse.tile as tile
import concourse.mybir as mybir
from concourse.bass_utils import run_bass_kernel_spmd

B, N, C, H = 2, 2048, 1024, 16
D = C // H          # 64
HL = H // 4         # 4 heads per core
CL = HL * D         # 256 local channels
N_CORES = 8
GROUPS = [[0, 1, 2, 3], [4, 5, 6, 7]]

F32 = mybir.dt.float32
BF16 = mybir.dt.float16
BF = np.float16

KC = C // 128       # 8  K-chunks of the input channel dim
NI = N // 512       # 4  512-wide i-chunks
NJ = N // 128       # 16 128-row j-chunks


def build_kernel(n_cores=N_CORES, groups=GROUPS):
    group_size = len(groups[0])
    out_rows = C // group_size          # 256 output channels per core

    nc = bacc.Bacc("TRN2", target_bir_lowering=False, debug=False,
                   num_devices=n_cores)

    xT = nc.declare_dram_parameter("xT", [C, N], BF16, isOutput=False)
    cos2 = nc.declare_dram_parameter("cos2", [128, N], BF16, isOutput=False)
    sin2s = nc.declare_dram_parameter("sin2s", [128, N], BF16, isOutput=False)
    wqkT = nc.declare_dram_parameter("wqkT", [C, 2 * CL], BF16, isOutput=False)
    bqk = nc.declare_dram_parameter("bqk", [2 * CL, 1], F32, isOutput=False)
    wvT = nc.declare_dram_parameter("wvT", [C, CL], BF16, isOutput=False)
    # out-projection weights, rows ordered (head_local, rank, d) to match
    # the AllGather concat order; cols = this core's 256 output channels
    wprojT = nc.declare_dram_parameter("wprojT", [C, out_rows], BF16,
                                       isOutput=False)
    beff = nc.declare_dram_parameter("beff", [out_rows, 1], F32, isOutput=False)
    out = nc.declare_dram_parameter("out", [out_rows, N], F32, isOutput=True)

    with tile.TileContext(nc) as tc:
        with tc.tile_pool(name="dram", bufs=1, space="DRAM") as dram, \
             tc.tile_pool(name="sbuf", bufs=1) as sb, \
             tc.tile_pool(name="psum", bufs=1, space="PSUM") as ps:

            # tile for clock-warming matmuls (see _warm_pe)
            warm = sb.tile([128, 128], BF16, name="warm", tag="warm")
            nc.vector.memset(warm[:], 0.001)

            def _warm_pe(tag, n):
                # short matmuls alternating two PSUM tiles: keeps the PE's
                # activity monitor busy so the clock gate stays at full rate
                wps = [ps.tile([128, 64], F32, name=f"warmp{tag}_{a}",
                               tag="sc", bufs=2) for a in range(2)]
                for r in range(n):
                    nc.tensor.matmul(wps[r % 2][:], warm[:], warm[:, :64],
                                     start=True, stop=True)

            _warm_pe("s", 16)

            # ---- input DMA: wqk first, then x in n-major column slices so
            # the kc-outer qk matmul chain for chunk n can start as soon as
            # that chunk's 8 column slices land ----
            wqk_sb, xb = [], []
            for kc in range(KC):
                t = sb.tile([128, 2 * CL], BF16, name=f"wqk{kc}", tag=f"wqk{kc}")
                eng = nc.scalar if kc % 2 == 0 else nc.sync
                eng.dma_start(t[:], wqkT.ap()[128 * kc:128 * (kc + 1), :])
                wqk_sb.append(t)
                xb.append(sb.tile([128, N], BF16, name=f"xb{kc}", tag=f"xb{kc}"))
            for n in range(NI):
                nsl = slice(512 * n, 512 * (n + 1))
                for kc in range(KC):
                    eng = nc.sync if (n * KC + kc) % 2 == 0 else nc.scalar
                    eng.dma_start(xb[kc][:, nsl],
                                  xT.ap()[128 * kc:128 * (kc + 1), nsl])
            cos_sb = sb.tile([128, N], BF16, name="cos_sb", tag="cos_sb")
            nc.sync.dma_start(cos_sb[:], cos2.ap())
            sin_sb = sb.tile([128, N], BF16, name="sin_sb", tag="sin_sb")
            nc.scalar.dma_start(sin_sb[:], sin2s.ap())
            wv_sb = []
            for kc in range(KC):
                t = sb.tile([128, CL], BF16, name=f"wv{kc}", tag=f"wv{kc}")
                eng = nc.sync if kc % 2 == 0 else nc.scalar
                eng.dma_start(t[:], wvT.ap()[128 * kc:128 * (kc + 1), :])
                wv_sb.append(t)
            bqk_sb = []
            for m in range(4):
                t = sb.tile([128, 1], F32, name=f"bqk{m}", tag=f"bqk{m}")
                nc.sync.dma_start(t[:], bqk.ap()[128 * m:128 * (m + 1), :])
                bqk_sb.append(t)
            wproj_sb = []
            for kc in range(KC):
                t = sb.tile([128, out_rows], BF16, name=f"wproj{kc}",
                            tag=f"wproj{kc}")
                nc.sync.dma_start(t[:], wprojT.ap()[128 * kc:128 * (kc + 1), :])
                wproj_sb.append(t)
            beff_sb = []
            for m in range(out_rows // 128):
                t = sb.tile([128, 1], F32, name=f"beff{m}", tag=f"beff{m}")
                nc.sync.dma_start(t[:], beff.ap()[128 * m:128 * (m + 1), :])
                beff_sb.append(t)

            # warm up the collective path during the preamble so the first
            # real gather doesn't absorb the first-collective cost
            agw_in = dram.tile([64, 8], BF16, name="agw_in", tag="agw_in")
            agw_out = dram.tile([64 * group_size, 8], BF16, name="agw_out",
                                tag="agw_out")
            agw_sb = sb.tile([64, 8], BF16, name="agw_sb", tag="agw_sb")
            nc.vector.memset(agw_sb[:], 0.0)
            nc.sync.dma_start(agw_in[:], agw_sb[:])
            nc.gpsimd.collective_compute(
                "AllGather", mybir.AluOpType.bypass, replica_groups=groups,
                ins=[agw_in[:]], outs=[agw_out[:]])

            # ---- qk projection + RoPE, per (m, n-chunk) job ----
            # chunk m rows: m=0:[q_h0,q_h1] m=1:[q_h2,q_h3] m=2:[k_h0,k_h1] m=3:[k_h2,k_h3]
            # so q and k of head h sit at the same partition offset 64*(h%2).
            # k of each head lands in its own zero-padded [128, N] tile so the
            # scores matmul can contract over K=128 (16-bit matmuls run at
            # half rate for K=64 -- zero rows buy back the full rate).
            k_t = []      # 4 tiles: k_h at rows 64*(h%2), zeros elsewhere
            for h in range(4):
                kt = sb.tile([128, N], BF16, name=f"ktile{h}", tag=f"ktile{h}")
                z = slice(0, 64) if h % 2 == 1 else slice(64, 128)
                nc.vector.memset(kt[z, :], 0.0)
                k_t.append(kt)
            swap_mask = [i ^ 1 for i in range(32)]
            qks_t = [sb.tile([128, N], BF16, name=f"qks{m}", tag=f"qks{m}")
                     for m in range(4)]
            q_r = [sb.tile([128, N], BF16, name=f"qkr{m}", tag=f"qkr{m}")
                   for m in range(2)]

            def qk_mm(n, m):
                # one projection job: accumulate [128, 512] over the 8 kc
                # chunks; returns the PSUM acc for eviction
                nsl = slice(512 * n, 512 * (n + 1))
                acc = ps.tile([128, 512], F32, name=f"qa{n}_{m}", tag="sc",
                              bufs=2)
                for kc in range(KC):
                    nc.tensor.matmul(acc[:],
                                     wqk_sb[kc][:, 128 * m:128 * (m + 1)],
                                     xb[kc][:, nsl],
                                     start=(kc == 0), stop=(kc == KC - 1))
                return acc

            def qk_finish(n, m, acc, eng):
                # bias-add eviction + RoPE for one (n, m) slice.
                # eng: ACT in the preamble (idle there), DVE when folded
                # into the attention stream (ACT is exp-bound there).
                nsl = slice(512 * n, 512 * (n + 1))
                qks = qks_t[m]
                if eng is nc.scalar:
                    nc.scalar.activation(
                        qks[:, nsl], acc[:],
                        mybir.ActivationFunctionType.Identity,
                        bias=bqk_sb[m][:])
                else:
                    nc.vector.tensor_scalar_add(qks[:, nsl], acc[:],
                                                bqk_sb[m][:])
                # RoPE: qk' = qks*cos2 + shift(qks)*sin2s
                # (pair-swap of adjacent partitions via DVE stream shuffle)
                shf = sb.tile([128, 512], BF16, name=f"shf{n}_{m}", tag="shf",
                              bufs=2)
                nc.vector.stream_shuffle(shf[:], qks[:, nsl], swap_mask)
                t2 = sb.tile([128, 512], BF16, name=f"rtmp{n}_{m}",
                             tag="ropetmp", bufs=2)
                nc.vector.tensor_mul(t2[:], shf[:], sin_sb[:, nsl])
                if m < 2:
                    qkr = q_r[m]
                    nc.vector.tensor_mul(qkr[:, nsl], qks[:, nsl],
                                         cos_sb[:, nsl])
                    nc.vector.tensor_add(qkr[:, nsl], qkr[:, nsl], t2[:])
                else:
                    t1 = sb.tile([128, 512], BF16, name=f"rtc{n}_{m}",
                                 tag="ropetc", bufs=2)
                    nc.vector.tensor_mul(t1[:], qks[:, nsl], cos_sb[:, nsl])
                    h0, h1 = 2 * (m - 2), 2 * (m - 2) + 1
                    nc.vector.tensor_add(k_t[h0][0:64, nsl], t1[0:64, :],
                                         t2[0:64, :])
                    nc.vector.tensor_add(k_t[h1][64:128, nsl], t1[64:128, :],
                                         t2[64:128, :])

            def qk_job(n, m, eng):
                qk_finish(n, m, qk_mm(n, m), eng)

            # pre-attention: k-proj for heads 0,1 (m2) and q-proj for the
            # first i-chunk's columns (m0 @ n0, n1), DMA-paced
            for n in range(NI):
                qk_job(n, 2, nc.scalar)
                if n < 2:
                    qk_job(n, 0, nc.scalar)

            # ---- v projection (natural [j, ch] layout, ones col per head) ----
            vaug = [None] * NJ

            def vproj_pair(jp, eng):
                jcs = (2 * jp, 2 * jp + 1)
                pvs = [ps.tile([128, CL], F32, name=f"pv{jc}", tag="sc",
                               bufs=2) for jc in jcs]
                for kc in range(KC):
                    for a, jc in enumerate(jcs):
                        nc.tensor.matmul(
                            pvs[a][:],
                            xb[kc][:, 128 * jc:128 * (jc + 1)],
                            wv_sb[kc][:],
                            start=(kc == 0), stop=(kc == KC - 1))
                for a, jc in enumerate(jcs):
                    va = sb.tile([128, HL * (D + 1)], BF16, name=f"vaug{jc}",
                                 tag=f"vaug{jc}")
                    nc.vector.memset(va[:, D::D + 1], 1.0)
                    if eng is nc.scalar:
                        nc.scalar.activation(
                            va.rearrange("p (h e) -> p h e", e=D + 1)[:, :, 0:D],
                            pvs[a].rearrange("p (h e) -> p h e", e=D)[:, :, :],
                            mybir.ActivationFunctionType.Copy)
                    else:
                        nc.vector.tensor_copy(
                            va.rearrange("p (h e) -> p h e", e=D + 1)[:, :, 0:D],
                            pvs[a].rearrange("p (h e) -> p h e", e=D)[:, :, :])
                    vaug[jc] = va

            # first 4 j-pairs before head 0 (vaug[0..7]); rest folded in
            for jp in range(4):
                vproj_pair(jp, nc.scalar)

            # per-partition bias AP used to shift scores before fp16 exp
            eshift = sb.tile([128, 1], F32, name="eshift", tag="eshift")
            nc.vector.memset(eshift[:], -16.0)
            # K=1 ones row used to broadcast denominators across partitions
            ones64 = sb.tile([1, 64], BF16, name="ones64", tag="ones64")
            nc.vector.memset(ones64[:], 1.0)

            # ---- attention, per i-chunk; per-head AllGather of the
            # normalized output; remaining projections and the previous
            # chunk's out-projection folded into the head pipelines ----
            chunks = [(0, 1024), (1024, 1024)]
            n_chunks = len(chunks)

            ag_in = [[dram.tile([64, cw], BF16, name=f"agin{ih}_{hl}",
                                tag=f"agin{ih}_{hl}")
                      for hl in range(4)] for ih, (i0, cw) in enumerate(chunks)]
            ag_out = [[dram.tile([64 * group_size, cw], BF16,
                                 name=f"agout{ih}_{hl}", tag=f"agout{ih}_{hl}")
                       for hl in range(4)] for ih, (i0, cw) in enumerate(chunks)]

            # AGs fired so far, in order; readbacks are emitted two head-slots
            # after the AG fires so a pending readback on the gpsimd queue
            # never sits between two AG triggers (that would serialize the
            # collective stream: AG k+1 couldn't trigger until AG k finished)
            ag_fired = []
            rb_state = {"done": 0}

            def drain_readbacks(limit=1):
                while len(ag_fired) - rb_state["done"] > limit:
                    fih, fhl, fcw = ag_fired[rb_state["done"]]
                    emit_readback(fih, fhl, fcw)
                    rb_state["done"] += 1

            def finalize_head(ih, hl, oacc, cw):
                # normalize: o[:, i] / den[i].  Broadcast den across
                # partitions with a K=1 matmul, then reciprocal+mul on 64
                # partitions; gather the per-head result across the group.
                den = sb.tile([1, cw], BF16, name=f"den{ih}_{hl}",
                              tag="den", bufs=2)
                nc.vector.tensor_copy(den[:], oacc[64:65, :])
                rb = ps.tile([64, cw], F32, name=f"rb{ih}_{hl}",
                             tag="oacc", bufs=2)
                for q in range(cw // 512):
                    nc.tensor.matmul(rb[:, 512 * q:512 * (q + 1)],
                                     ones64[:],
                                     den[:, 512 * q:512 * (q + 1)],
                                     start=True, stop=True)
                rr = sb.tile([64, cw], F32, name=f"rr{ih}_{hl}", tag="rr",
                             bufs=2)
                nc.vector.reciprocal_approx_fast(rr[:], rb[:])
                oh = sb.tile([64, cw], BF16, name=f"oh{ih}_{hl}", tag="oh",
                             bufs=2)
                nc.vector.tensor_mul(oh[:], oacc[0:64, :], rr[:])
                nc.sync.dma_start(ag_in[ih][hl][:], oh[:])
                nc.gpsimd.collective_compute(
                    "AllGather", mybir.AluOpType.bypass,
                    replica_groups=groups,
                    ins=[ag_in[ih][hl][:]],
                    outs=[ag_out[ih][hl][:]])
                ag_fired.append((ih, hl, cw))

            # gathered o readback tiles, kc = hl*2 + half (row order matches
            # wprojT's (head_local, rank, d) ordering)
            o_rb = [[None] * KC for _ in range(n_chunks)]

            def emit_readback(ih, hl, cw):
                for half in range(2):
                    t = sb.tile([128, cw], BF16, name=f"orb{ih}_{hl}_{half}",
                                tag="orb", bufs=12)
                    nc.gpsimd.dma_start(
                        t[:], ag_out[ih][hl][128 * half:128 * (half + 1), :])
                    o_rb[ih][2 * hl + half] = t

            def proj_mm(ih, mc, n2, i0, pp, kcs, start, stop):
                isl = slice(512 * n2, 512 * (n2 + 1))
                for kc in kcs:
                    nc.tensor.matmul(
                        pp[:],
                        wproj_sb[kc][:, 128 * mc:128 * (mc + 1)],
                        o_rb[ih][kc][:, isl],
                        start=start and kc == kcs[0],
                        stop=stop and kc == kcs[-1])

            def proj_evict(ih, mc, n2, i0, pp):
                fo = sb.tile([128, 512], F32, name=f"fo{ih}_{mc}_{n2}",
                             tag="fo", bufs=4)
                nc.vector.tensor_scalar_add(fo[:], pp[:], beff_sb[mc][:])
                nc.sync.dma_start(
                    out.ap()[128 * mc:128 * (mc + 1),
                             i0 + 512 * n2:i0 + 512 * (n2 + 1)], fo[:])

            def emit_proj_job(ih, mc, n2, i0, cw):
                pp = ps.tile([128, 512], F32, name=f"pp{ih}_{mc}_{n2}",
                             tag="sc", bufs=2)
                proj_mm(ih, mc, n2, i0, pp, list(range(KC)), True, True)
                proj_evict(ih, mc, n2, i0, pp)

            # fold schedule: (ih, hl, jc) -> list of thunks to emit inside
            # the attention pipeline at that point (PE slack)
            folds = {}

            def add_fold(ih, hl, jc, fn):
                folds.setdefault((ih, hl, jc), []).append(fn)

            # remaining v-projection: vaug[8..15] consumed from head0 jc9
            add_fold(0, 0, 1, lambda: vproj_pair(4, nc.vector))
            add_fold(0, 0, 4, lambda: vproj_pair(5, nc.vector))
            add_fold(0, 0, 7, lambda: vproj_pair(6, nc.vector))
            add_fold(0, 0, 10, lambda: vproj_pair(7, nc.vector))
            # k-proj heads 2,3 (m3, all n): needed by (0, hl2) scores
            for n in range(NI):
                add_fold(0, 1, 1 + 3 * n,
                         lambda n=n: qk_job(n, 3, nc.vector))
            # q-proj heads 2,3 first half (m1 @ n0,n1): head 2's scores read
            # q_r[1][:, 0:1024] from jc0, so both must be emitted (and thus
            # ordered) before head 2 starts
            add_fold(0, 1, 13, lambda: qk_job(0, 1, nc.vector))
            add_fold(0, 1, 15, lambda: qk_job(1, 1, nc.vector))
            # q-proj second halves, needed by chunk 1 (heads 0,1 and 2,3)
            add_fold(0, 2, 2, lambda: qk_job(2, 0, nc.vector))
            add_fold(0, 2, 8, lambda: qk_job(3, 0, nc.vector))
            add_fold(0, 3, 2, lambda: qk_job(2, 1, nc.vector))
            add_fold(0, 3, 8, lambda: qk_job(3, 1, nc.vector))

            for ih, (i0, cw) in enumerate(chunks):
                ns = cw // 512
                # out-projection quadrants of the previous chunk, folded
                # into heads 2,3 (the last gather lands mid-head-1)
                if ih > 0:
                    pi0, pcw = chunks[ih - 1]
                    jobs = [(ih - 1, mc, n2, pi0, pcw)
                            for mc in range(out_rows // 128)
                            for n2 in range(pcw // 512)]
                    add_fold(ih, 2, 2, lambda j=jobs[0]: emit_proj_job(*j))
                    add_fold(ih, 2, 8, lambda j=jobs[1]: emit_proj_job(*j))
                    add_fold(ih, 3, 2, lambda j=jobs[2]: emit_proj_job(*j))
                    add_fold(ih, 3, 8, lambda j=jobs[3]: emit_proj_job(*j))
                pending = None
                for hl in range(4):
                    qT = q_r[hl // 2]
                    kT = k_t[hl]
                    oacc = ps.tile([65, cw], F32, name=f"oacc{ih}_{hl}",
                                   tag="oacc", bufs=2)
                    exs = []

                    def emit_o(jc, oacc=oacc, exs=exs, hl=hl):
                        for q in range(ns):
                            nc.tensor.matmul(
                                oacc[:, 512 * q:512 * (q + 1)],
                                vaug[jc][:, (D + 1) * hl:(D + 1) * (hl + 1)],
                                exs[jc][:, 512 * q:512 * (q + 1)],
                                start=(jc == 0), stop=(jc == NJ - 1))

                    for jc in range(NJ):
                        sc = ps.tile([128, cw], F32, name=f"sc{ih}_{hl}_{jc}",
                                     tag="sc", bufs=2)
                        for q in range(ns):
                            nc.tensor.matmul(
                                sc[:, 512 * q:512 * (q + 1)],
                                kT[:, 128 * jc:128 * (jc + 1)],
                                qT[:, i0 + 512 * q:i0 + 512 * (q + 1)],
                                start=True, stop=True)
                        ex = sb.tile([128, cw], BF16, name=f"ex{ih}_{hl}_{jc}",
                                     tag="ex", bufs=3)
                        # bias shifts all scores so fp16 exp can't overflow
                        # (softmax is shift-invariant, cancels in num/den)
                        nc.scalar.activation(ex[:], sc[:],
                                             mybir.ActivationFunctionType.Exp,
                                             scale=float(1.0 / np.sqrt(D)),
                                             bias=eshift[:])
                        exs.append(ex)
                        # software pipeline: o-matmuls lag one j-chunk, and
                        # the previous head's normalization chain is deferred
                        # into this head's pipeline so the exp stream never
                        # pauses at head boundaries
                        if jc == 1 and pending is not None:
                            finalize_head(*pending)
                            pending = None
                        for fn in folds.pop((ih, hl, jc), ()):
                            fn()
                        if jc >= 1:
                            emit_o(jc - 1)
                    emit_o(NJ - 1)
                    pending = (ih, hl, oacc, cw)
                    # emit readbacks for gathers fired two head-slots ago
                    drain_readbacks(limit=1)
                # keep the PE clock-gate warm through the last head's
                # normalization chain so the tail starts at full rate
                _warm_pe(f"p{ih}", 12)
                finalize_head(*pending)
                pending = None

            # ---- tail: last chunk's out-projection.  kc 0..5 (heads 0-2,
            # gathers already landed) run during the final gather; kc 6,7
            # finish once it lands ----
            drain_readbacks(limit=0)
            li0, lcw = chunks[-1]
            tail_jobs = [(n_chunks - 1, mc, n2, li0, lcw)
                         for mc in range(out_rows // 128)
                         for n2 in range(lcw // 512)]
            pps = []
            for j, (ih, mc, n2, i0, cw) in enumerate(tail_jobs):
                pp = ps.tile([128, 512], F32, name=f"tpp{mc}_{n2}",
                             tag="sc" if j < 2 else "oacc", bufs=2)
                proj_mm(ih, mc, n2, i0, pp, list(range(6)), True, False)
                pps.append(pp)
            for j, (ih, mc, n2, i0, cw) in enumerate(tail_jobs):
                proj_mm(ih, mc, n2, i0, pps[j], [6, 7], False, True)
                proj_evict(ih, mc, n2, i0, pps[j])

    nc.compile()
    return nc


def shard_inputs(x, rope, w_qkv, b_qkv, w_proj, b_proj,
                 n_cores=N_CORES, group_size=4):
    """Per-core input maps. Host-side transposes/casts are part of sharding."""
    out_rows = C // group_size
    # fold the v-bias through the projection into an effective output bias
    b_v = b_qkv[2 * C:3 * C]
    b_eff = (b_proj + b_v @ w_proj.T).astype(np.float32)   # [C]

    in_maps = []
    for c in range(n_cores):
        b = (c // group_size) % B
        g = c % group_size
        heads = range(HL * g, HL * g + HL)

        xTb = np.ascontiguousarray(x[b].T).astype(BF)            # [C, N]

        cosT = rope[b].T[:D, :]                                   # [64, N]
        sinT = rope[b].T[D:, :]
        cos2 = np.vstack([cosT, cosT]).astype(BF)                 # [128, N]
        sgn = np.where(np.arange(128) % 2 == 0, -1.0, 1.0)[:, None]
        sin2s = (np.vstack([sinT, sinT]) * sgn).astype(BF)        # [128, N]

        # qk weight rows ordered [q_h0..q_h3, k_h0..k_h3]
        qk_rows = []
        bqk_rows = []
        for h in heads:
            qk_rows.append(w_qkv[D * h:D * (h + 1), :])           # q rows
            bqk_rows.append(b_qkv[D * h:D * (h + 1)])
        for h in heads:
            qk_rows.append(w_qkv[C + D * h:C + D * (h + 1), :])   # k rows
            bqk_rows.append(b_qkv[C + D * h:C + D * (h + 1)])
        wqk = np.vstack(qk_rows)                                  # [512, C]
        wqkT = np.ascontiguousarray(wqk.T).astype(BF)             # [C, 512]
        bqk_v = np.concatenate(bqk_rows).astype(np.float32)[:, None]

        h0 = HL * g
        wv = w_qkv[2 * C + D * h0:2 * C + D * h0 + CL, :]          # [256, C]
        wvT = np.ascontiguousarray(wv.T).astype(BF)                # [C, 256]

        # out-projection: this core owns output channels
        # [out_rows*g, out_rows*(g+1)).  The contraction rows are ordered
        # (head_local hl, rank r, d) to match the per-head AllGather layout
        # (gather hl concatenates rank blocks of 64 rows).
        osl = slice(out_rows * g, out_rows * (g + 1))
        wp_rows = []
        for hl in range(HL):
            for r in range(group_size):
                gh = HL * r + hl                   # global head of (r, hl)
                wp_rows.append(w_proj[osl, D * gh:D * (gh + 1)].T)  # [64,256]
        wprojT = np.ascontiguousarray(np.vstack(wp_rows)).astype(BF)  # [C,256]

        beff_shard = b_eff[osl].astype(np.float32)[:, None]

        in_maps.append({
            "xT": xTb, "cos2": cos2, "sin2s": sin2s,
            "wqkT": wqkT, "bqk": bqk_v, "wvT": wvT,
            "wprojT": wprojT, "beff": beff_shard,
        })
    return in_maps


def assemble(results, n_cores=N_CORES, group_size=4):
    out_rows = C // group_size
    out = np.empty((B, N, C), dtype=np.float32)
    for c in range(n_cores):
        b = (c // group_size) % B
        g = c % group_size
        outT_shard = results[c]["out"]                 # [out_rows, N] f32
        out[b, :, out_rows * g:out_rows * (g + 1)] = outT_shard.T
    return out


_NC_CACHE = {}


def _get_nc():
    if "nc" not in _NC_CACHE:
        _NC_CACHE["nc"] = build_kernel()
    return _NC_CACHE["nc"]


def _run(inputs, trace=False, tmpdir=None):
    nc = _get_nc()
    inputs = {k: np.asarray(v) for k, v in inputs.items()}
    in_maps = shard_inputs(**inputs)
    res = run_bass_kernel_spmd(nc, in_maps, core_ids=list(range(N_CORES)),
                               trace=trace, tmpdir=tmpdir)
    return assemble(res.results), res


def kernel(**inputs):
    out, _ = _run(inputs)
    return out


# revision 20
# speedup vs baseline: 1.1008x; 1.0058x over previous
"""Multi-head attention (B=2, N=2048, C=1024, H=16, D=64) on 8 TRN2 NeuronCores.

Sharding: core c = (batch b = c//4) x (head-group g = c%4 -> heads 4g..4g+3).
Data parallel on B, tensor parallel on heads.  After each head's softmax
normalization, the [64, cw] head-output is AllGathered (fp16) within the
4-core batch group; each core then runs the full-K (1024) out-projection
for its 256-channel output slice locally -- no reduce, no post-projection
collective, and the gathers overlap the attention pipeline.

The projections are software-pipelined into the attention stream: only the
k-projection (m2) and the first half of the q-projection (m0 @ n0,n1) run
before head 0 starts; the remaining qk projection chunks, the whole
v-projection and the previous chunk's out-projection quadrants are folded
into the attention pipeline's PE slack (the inner loop is ACT/exp-bound).

Everything on device stays transposed ([channel, position]); the host
pre-transposes inputs and post-transposes the output.
"""

import numpy as np

import concourse.bacc as bacc
import concourse.tile as tile
import concourse.mybir as mybir
from concourse.bass_utils import run_bass_kernel_spmd

B, N, C, H = 2, 2048, 1024, 16
D = C // H          # 64
HL = H // 4         # 4 heads per core
CL = HL * D         # 256 local channels
N_CORES = 8
GROUPS = [[0, 1, 2, 3], [4, 5, 6, 7]]

F32 = mybir.dt.float32
BF16 = mybir.dt.float16
BF = np.float16

KC = C // 128       # 8  K-chunks of the input channel dim
NI = N // 512       # 4  512-wide i-chunks
NJ = N // 128       # 16 128-row j-chunks


def build_kernel(n_cores=N_CORES, groups=GROUPS):
    group_size = len(groups[0])
    out_rows = C // group_size          # 256 output channels per core

    nc = bacc.Bacc("TRN2", target_bir_lowering=False, debug=False,
                   num_devices=n_cores)

    xT = nc.declare_dram_parameter("xT", [C, N], BF16, isOutput=False)
    cos2 = nc.declare_dram_parameter("cos2", [128, N], BF16, isOutput=False)
    sin2s = nc.declare_dram_parameter("sin2s", [128, N], BF16, isOutput=False)
    wqkT = nc.declare_dram_parameter("wqkT", [C, 2 * CL], BF16, isOutput=False)
    bqk = nc.declare_dram_parameter("bqk", [2 * CL, 1], F32, isOutput=False)
    wvT = nc.declare_dram_parameter("wvT", [C, CL], BF16, isOutput=False)
    # out-projection weights, rows ordered (head_local, rank, d) to match
    # the AllGather concat order; cols = this core's 256 output channels
    wprojT = nc.declare_dram_parameter("wprojT", [C, out_rows], BF16,
                                       isOutput=False)
    beff = nc.declare_dram_parameter("beff", [out_rows, 1], F32, isOutput=False)
    out = nc.declare_dram_parameter("out", [out_rows, N], F32, isOutput=True)

    with tile.TileContext(nc) as tc:
        with tc.tile_pool(name="dram", bufs=1, space="DRAM") as dram, \
             tc.tile_pool(name="sbuf", bufs=1) as sb, \
             tc.tile_pool(name="psum", bufs=1, space="PSUM") as ps:

            # tile for clock-warming matmuls (see _warm_pe)
            warm = sb.tile([128, 128], BF16, name="warm", tag="warm")
            nc.vector.memset(warm[:], 0.001)

            def _warm_pe(tag, n):
                # short matmuls alternating two PSUM tiles: keeps the PE's
                # activity monitor busy so the clock gate stays at full rate
                wps = [ps.tile([128, 64], F32, name=f"warmp{tag}_{a}",
                               tag="sc", bufs=2) for a in range(2)]
                for r in range(n):
                    nc.tensor.matmul(wps[r % 2][:], warm[:], warm[:, :64],
                                     start=True, stop=True)

            _warm_pe("s", 16)

            # ---- input DMA: wqk first, then x in n-major column slices so
            # the kc-outer qk matmul chain for chunk n can start as soon as
            # that chunk's 8 column slices land ----
            wqk_sb, xb = [], []
            for kc in range(KC):
                t = sb.tile([128, 2 * CL], BF16, name=f"wqk{kc}", tag=f"wqk{kc}")
                eng = nc.scalar if kc % 2 == 0 else nc.sync
                eng.dma_start(t[:], wqkT.ap()[128 * kc:128 * (kc + 1), :])
                wqk_sb.append(t)
                t = sb.tile([128, N], BF16, name=f"xb{kc}", tag=f"xb{kc}")
                eng = nc.sync if kc % 2 == 0 else nc.scalar
                eng.dma_start(t[:], xT.ap()[128 * kc:128 * (kc + 1), :])
                xb.append(t)
            cos_sb = sb.tile([128, N], BF16, name="cos_sb", tag="cos_sb")
            nc.sync.dma_start(cos_sb[:], cos2.ap())
            sin_sb = sb.tile([128, N], BF16, name="sin_sb", tag="sin_sb")
            nc.scalar.dma_start(sin_sb[:], sin2s.ap())
            wv_sb = []
            for kc in range(KC):
                t = sb.tile([128, CL], BF16, name=f"wv{kc}", tag=f"wv{kc}")
                eng = nc.sync if kc % 2 == 0 else nc.scalar
                eng.dma_start(t[:], wvT.ap()[128 * kc:128 * (kc + 1), :])
                wv_sb.append(t)
            bqk_sb = []
            for m in range(4):
                t = sb.tile([128, 1], F32, name=f"bqk{m}", tag=f"bqk{m}")
                nc.sync.dma_start(t[:], bqk.ap()[128 * m:128 * (m + 1), :])
                bqk_sb.append(t)
            wproj_sb = []
            for kc in range(KC):
                t = sb.tile([128, out_rows], BF16, name=f"wproj{kc}",
                            tag=f"wproj{kc}")
                nc.sync.dma_start(t[:], wprojT.ap()[128 * kc:128 * (kc + 1), :])
                wproj_sb.append(t)
            beff_sb = []
            for m in range(out_rows // 128):
                t = sb.tile([128, 1], F32, name=f"beff{m}", tag=f"beff{m}")
                nc.sync.dma_start(t[:], beff.ap()[128 * m:128 * (m + 1), :])
                beff_sb.append(t)

            # warm up the collective path during the preamble so the first
            # real gather doesn't absorb the first-collective cost
            agw_in = dram.tile([64, 8], BF16, name="agw_in", tag="agw_in")
            agw_out = dram.tile([64 * group_size, 8], BF16, name="agw_out",
                                tag="agw_out")
            agw_sb = sb.tile([64, 8], BF16, name="agw_sb", tag="agw_sb")
            nc.vector.memset(agw_sb[:], 0.0)
            nc.sync.dma_start(agw_in[:], agw_sb[:])
            nc.gpsimd.collective_compute(
                "AllGather", mybir.AluOpType.bypass, replica_groups=groups,
                ins=[agw_in[:]], outs=[agw_out[:]])

            # ---- qk projection + RoPE, per (m, n-chunk) job ----
            # chunk m rows: m=0:[q_h0,q_h1] m=1:[q_h2,q_h3] m=2:[k_h0,k_h1] m=3:[k_h2,k_h3]
            # so q and k of head h sit at the same partition offset 64*(h%2).
            # k of each head lands in its own zero-padded [128, N] tile so the
            # scores matmul can contract over K=128 (16-bit matmuls run at
            # half rate for K=64 -- zero rows buy back the full rate).
            k_t = []      # 4 tiles: k_h at rows 64*(h%2), zeros elsewhere
            for h in range(4):
                kt = sb.tile([128, N], BF16, name=f"ktile{h}", tag=f"ktile{h}")
                z = slice(0, 64) if h % 2 == 1 else slice(64, 128)
                nc.vector.memset(kt[z, :], 0.0)
                k_t.append(kt)
            swap_mask = [i ^ 1 for i in range(32)]
            qks_t = [sb.tile([128, N], BF16, name=f"qks{m}", tag=f"qks{m}")
                     for m in range(4)]
            q_r = [sb.tile([128, N], BF16, name=f"qkr{m}", tag=f"qkr{m}")
                   for m in range(2)]

            def qk_mm(n, m):
                # one projection job: accumulate [128, 512] over the 8 kc
                # chunks; returns the PSUM acc for eviction
                nsl = slice(512 * n, 512 * (n + 1))
                acc = ps.tile([128, 512], F32, name=f"qa{n}_{m}", tag="sc",
                              bufs=2)
                for kc in range(KC):
                    nc.tensor.matmul(acc[:],
                                     wqk_sb[kc][:, 128 * m:128 * (m + 1)],
                                     xb[kc][:, nsl],
                                     start=(kc == 0), stop=(kc == KC - 1))
                return acc

            def qk_finish(n, m, acc):
                # bias-add eviction (ACT; cheap enough to ride the exp
                # stream when folded -- a DVE eviction queues behind the
                # finalize chain and stalls the sc-slot rotation) + RoPE
                nsl = slice(512 * n, 512 * (n + 1))
                qks = qks_t[m]
                nc.scalar.activation(
                    qks[:, nsl], acc[:],
                    mybir.ActivationFunctionType.Identity,
                    bias=bqk_sb[m][:])
                # RoPE: qk' = qks*cos2 + shift(qks)*sin2s
                # (pair-swap of adjacent partitions via DVE stream shuffle)
                shf = sb.tile([128, 512], BF16, name=f"shf{n}_{m}", tag="shf",
                              bufs=2)
                nc.vector.stream_shuffle(shf[:], qks[:, nsl], swap_mask)
                t2 = sb.tile([128, 512], BF16, name=f"rtmp{n}_{m}",
                             tag="ropetmp", bufs=2)
                nc.vector.tensor_mul(t2[:], shf[:], sin_sb[:, nsl])
                if m < 2:
                    qkr = q_r[m]
                    nc.vector.tensor_mul(qkr[:, nsl], qks[:, nsl],
                                         cos_sb[:, nsl])
                    nc.vector.tensor_add(qkr[:, nsl], qkr[:, nsl], t2[:])
                else:
                    t1 = sb.tile([128, 512], BF16, name=f"rtc{n}_{m}",
                                 tag="ropetc", bufs=2)
                    nc.vector.tensor_mul(t1[:], qks[:, nsl], cos_sb[:, nsl])
                    h0, h1 = 2 * (m - 2), 2 * (m - 2) + 1
                    nc.vector.tensor_add(k_t[h0][0:64, nsl], t1[0:64, :],
                                         t2[0:64, :])
                    nc.vector.tensor_add(k_t[h1][64:128, nsl], t1[64:128, :],
                                         t2[64:128, :])

            def qk_job(n, m):
                qk_finish(n, m, qk_mm(n, m))

            # pre-attention: k-proj for heads 0,1 (m2) and q-proj for the
            # first i-chunk's columns (m0 @ n0, n1), DMA-paced
            for n in range(NI):
                qk_job(n, 2)
                if n < 2:
                    qk_job(n, 0)

            # ---- v projection (natural [j, ch] layout, ones col per head) ----
            vaug = [None] * NJ

            def vproj_pair(jp):
                jcs = (2 * jp, 2 * jp + 1)
                pvs = [ps.tile([128, CL], F32, name=f"pv{jc}", tag="sc",
                               bufs=2) for jc in jcs]
                for kc in range(KC):
                    for a, jc in enumerate(jcs):
                        nc.tensor.matmul(
                            pvs[a][:],
                            xb[kc][:, 128 * jc:128 * (jc + 1)],
                            wv_sb[kc][:],
                            start=(kc == 0), stop=(kc == KC - 1))
                for a, jc in enumerate(jcs):
                    va = sb.tile([128, HL * (D + 1)], BF16, name=f"vaug{jc}",
                                 tag=f"vaug{jc}")
                    nc.vector.memset(va[:, D::D + 1], 1.0)
                    nc.scalar.activation(
                        va.rearrange("p (h e) -> p h e", e=D + 1)[:, :, 0:D],
                        pvs[a].rearrange("p (h e) -> p h e", e=D)[:, :, :],
                        mybir.ActivationFunctionType.Copy)
                    vaug[jc] = va

            # first 4 j-pairs before head 0 (vaug[0..7]); rest folded in
            for jp in range(4):
                vproj_pair(jp)

            # per-partition bias AP used to shift scores before fp16 exp
            eshift = sb.tile([128, 1], F32, name="eshift", tag="eshift")
            nc.vector.memset(eshift[:], -16.0)
            # K=1 ones row used to broadcast denominators across partitions
            ones64 = sb.tile([1, 64], BF16, name="ones64", tag="ones64")
            nc.vector.memset(ones64[:], 1.0)

            # ---- attention, per i-chunk; per-head AllGather of the
            # normalized output; remaining projections and the previous
            # chunk's out-projection folded into the head pipelines ----
            chunks = [(0, 1024), (1024, 1024)]
            n_chunks = len(chunks)

            ag_in = [[dram.tile([64, cw], BF16, name=f"agin{ih}_{hl}",
                                tag=f"agin{ih}_{hl}")
                      for hl in range(4)] for ih, (i0, cw) in enumerate(chunks)]
            ag_out = [[dram.tile([64 * group_size, cw], BF16,
                                 name=f"agout{ih}_{hl}", tag=f"agout{ih}_{hl}")
                       for hl in range(4)] for ih, (i0, cw) in enumerate(chunks)]

            # AGs fired so far, in order; readbacks are emitted two head-slots
            # after the AG fires so a pending readback on the gpsimd queue
            # never sits between two AG triggers (that would serialize the
            # collective stream: AG k+1 couldn't trigger until AG k finished)
            ag_fired = []
            rb_state = {"done": 0}

            def drain_readbacks(limit=1):
                while len(ag_fired) - rb_state["done"] > limit:
                    fih, fhl, fcw = ag_fired[rb_state["done"]]
                    emit_readback(fih, fhl, fcw)
                    rb_state["done"] += 1

            def finalize_head(ih, hl, oacc, cw):
                # normalize: o[:, i] / den[i].  Broadcast den across
                # partitions with a K=1 matmul, then reciprocal+mul on 64
                # partitions; gather the per-head result across the group.
                den = sb.tile([1, cw], BF16, name=f"den{ih}_{hl}",
                              tag="den", bufs=2)
                nc.vector.tensor_copy(den[:], oacc[64:65, :])
                rb = ps.tile([64, cw], F32, name=f"rb{ih}_{hl}",
                             tag="oacc", bufs=2)
                for q in range(cw // 512):
                    nc.tensor.matmul(rb[:, 512 * q:512 * (q + 1)],
                                     ones64[:],
                                     den[:, 512 * q:512 * (q + 1)],
                                     start=True, stop=True)
                rr = sb.tile([64, cw], F32, name=f"rr{ih}_{hl}", tag="rr",
                             bufs=2)
                nc.vector.reciprocal_approx_fast(rr[:], rb[:])
                oh = sb.tile([64, cw], BF16, name=f"oh{ih}_{hl}", tag="oh",
                             bufs=2)
                nc.vector.tensor_mul(oh[:], oacc[0:64, :], rr[:])
                nc.sync.dma_start(ag_in[ih][hl][:], oh[:])
                nc.gpsimd.collective_compute(
                    "AllGather", mybir.AluOpType.bypass,
                    replica_groups=groups,
                    ins=[ag_in[ih][hl][:]],
                    outs=[ag_out[ih][hl][:]])
                ag_fired.append((ih, hl, cw))

            # gathered o readback tiles, kc = hl*2 + half (row order matches
            # wprojT's (head_local, rank, d) ordering)
            o_rb = [[None] * KC for _ in range(n_chunks)]

            def emit_readback(ih, hl, cw):
                for half in range(2):
                    t = sb.tile([128, cw], BF16, name=f"orb{ih}_{hl}_{half}",
                                tag="orb", bufs=12)
                    nc.gpsimd.dma_start(
                        t[:], ag_out[ih][hl][128 * half:128 * (half + 1), :])
                    o_rb[ih][2 * hl + half] = t

            def proj_mm(ih, mc, n2, i0, pp, kcs, start, stop):
                isl = slice(512 * n2, 512 * (n2 + 1))
                for kc in kcs:
                    nc.tensor.matmul(
                        pp[:],
                        wproj_sb[kc][:, 128 * mc:128 * (mc + 1)],
                        o_rb[ih][kc][:, isl],
                        start=start and kc == kcs[0],
                        stop=stop and kc == kcs[-1])

            def proj_evict(ih, mc, n2, i0, pp):
                fo = sb.tile([128, 512], F32, name=f"fo{ih}_{mc}_{n2}",
                             tag="fo", bufs=4)
                nc.vector.tensor_scalar_add(fo[:], pp[:], beff_sb[mc][:])
                nc.sync.dma_start(
                    out.ap()[128 * mc:128 * (mc + 1),
                             i0 + 512 * n2:i0 + 512 * (n2 + 1)], fo[:])

            def emit_proj_job(ih, mc, n2, i0, cw):
                pp = ps.tile([128, 512], F32, name=f"pp{ih}_{mc}_{n2}",
                             tag="sc", bufs=2)
                proj_mm(ih, mc, n2, i0, pp, list(range(KC)), True, True)
                proj_evict(ih, mc, n2, i0, pp)

            # fold schedule: (ih, hl, jc) -> list of thunks to emit inside
            # the attention pipeline at that point (PE slack)
            folds = {}

            def add_fold(ih, hl, jc, fn):
                folds.setdefault((ih, hl, jc), []).append(fn)

            # remaining v-projection: vaug[8..15] consumed from head0 jc9
            add_fold(0, 0, 1, lambda: vproj_pair(4))
            add_fold(0, 0, 4, lambda: vproj_pair(5))
            add_fold(0, 0, 7, lambda: vproj_pair(6))
            add_fold(0, 0, 10, lambda: vproj_pair(7))
            # k-proj heads 2,3 (m3): k_t[2] read from head 2, k_t[3] from
            # head 3, so m3@n3 can slip to head 2.  q-proj m1 @ n0,n1 must
            # be emitted before head 2 (its scores read q_r[1][:, 0:1024]
            # from jc0); m1/m0 second halves before chunk 1.
            add_fold(0, 1, 2, lambda: qk_job(0, 3))
            add_fold(0, 1, 6, lambda: qk_job(1, 3))
            add_fold(0, 1, 10, lambda: qk_job(2, 3))
            add_fold(0, 1, 12, lambda: qk_job(0, 1))
            add_fold(0, 1, 14, lambda: qk_job(1, 1))
            add_fold(0, 2, 2, lambda: qk_job(3, 3))
            add_fold(0, 2, 5, lambda: qk_job(2, 0))
            add_fold(0, 2, 8, lambda: qk_job(3, 0))
            add_fold(0, 3, 2, lambda: qk_job(2, 1))
            add_fold(0, 3, 8, lambda: qk_job(3, 1))

            for ih, (i0, cw) in enumerate(chunks):
                ns = cw // 512
                # out-projection quadrants of the previous chunk, folded
                # into heads 2,3 (the last gather lands mid-head-1)
                if ih > 0:
                    pi0, pcw = chunks[ih - 1]
                    jobs = [(ih - 1, mc, n2, pi0, pcw)
                            for mc in range(out_rows // 128)
                            for n2 in range(pcw // 512)]
                    add_fold(ih, 2, 2, lambda j=jobs[0]: emit_proj_job(*j))
                    add_fold(ih, 2, 8, lambda j=jobs[1]: emit_proj_job(*j))
                    add_fold(ih, 3, 2, lambda j=jobs[2]: emit_proj_job(*j))
                    add_fold(ih, 3, 8, lambda j=jobs[3]: emit_proj_job(*j))
                pending = None
                for hl in range(4):
                    qT = q_r[hl // 2]
                    kT = k_t[hl]
                    oacc = ps.tile([65, cw], F32, name=f"oacc{ih}_{hl}",
                                   tag="oacc", bufs=2)
                    exs = []

                    def emit_o(jc, oacc=oacc, exs=exs, hl=hl):
                        for q in range(ns):
                            nc.tensor.matmul(
                                oacc[:, 512 * q:512 * (q + 1)],
                                vaug[jc][:, (D + 1) * hl:(D + 1) * (hl + 1)],
                                exs[jc][:, 512 * q:512 * (q + 1)],
                                start=(jc == 0), stop=(jc == NJ - 1))

                    for jc in range(NJ):
                        sc = ps.tile([128, cw], F32, name=f"sc{ih}_{hl}_{jc}",
                                     tag="sc", bufs=2)
                        for q in range(ns):
                            nc.tensor.matmul(
                                sc[:, 512 * q:512 * (q + 1)],
                                kT[:, 128 * jc:128 * (jc + 1)],
                                qT[:, i0 + 512 * q:i0 + 512 * (q + 1)],
                                start=True, stop=True)
                        ex = sb.tile([128, cw], BF16, name=f"ex{ih}_{hl}_{jc}",
                                     tag="ex", bufs=3)
                        # bias shifts all scores so fp16 exp can't overflow
                        # (softmax is shift-invariant, cancels in num/den)
                        nc.scalar.activation(ex[:], sc[:],
                                             mybir.ActivationFunctionType.Exp,
                                             scale=float(1.0 / np.sqrt(D)),
                                             bias=eshift[:])
                        exs.append(ex)
                        # software pipeline: o-matmuls lag one j-chunk, and
                        # the previous head's normalization chain is deferred
                        # into this head's pipeline so the exp stream never
                        # pauses at head boundaries
                        if jc == 1 and pending is not None:
                            finalize_head(*pending)
                            pending = None
                        for fn in folds.pop((ih, hl, jc), ()):
                            fn()
                        if jc >= 1:
                            emit_o(jc - 1)
                    emit_o(NJ - 1)
                    pending = (ih, hl, oacc, cw)
                    # emit readbacks for gathers fired two head-slots ago
                    drain_readbacks(limit=1)
                # keep the PE clock-gate warm through the last head's
                # normalization chain so the tail starts at full rate
                _warm_pe(f"p{ih}", 12)
                if ih < n_chunks - 1:
                    finalize_head(*pending)
                    pending = None

            # ---- tail ----
            # the last head's finalize is split into 512-column halves, each
            # with its own gather, pipelining the den/recip/mul chain and
            # halving the exposed collective latency; den copies ride the
            # now-idle ACT engine
            drain_readbacks(limit=0)
            _, _, loacc, _ = pending
            li0, lcw = chunks[-1]
            ag_in2 = [dram.tile([64, 512], BF16, name=f"agin2_{h}",
                                tag=f"agin2_{h}") for h in range(2)]
            ag_out2 = [dram.tile([64 * group_size, 512], BF16,
                                 name=f"agout2_{h}", tag=f"agout2_{h}")
                       for h in range(2)]
            for half in range(2):
                hsl = slice(512 * half, 512 * (half + 1))
                den = sb.tile([1, 512], BF16, name=f"dent{half}", tag="den",
                              bufs=2)
                nc.scalar.activation(den[:], loacc[64:65, hsl],
                                     mybir.ActivationFunctionType.Copy)
                rbt = ps.tile([64, 512], F32, name=f"rbt{half}",
                              tag="oacc" if half == 0 else "sc", bufs=2)
                nc.tensor.matmul(rbt[:], ones64[:], den[:],
                                 start=True, stop=True)
                rr = sb.tile([64, 512], F32, name=f"rrt{half}", tag="rr",
                             bufs=2)
                nc.vector.reciprocal_approx_fast(rr[:], rbt[:])
                oht = sb.tile([64, 512], BF16, name=f"oht{half}", tag="oh",
                              bufs=2)
                nc.vector.tensor_mul(oht[:], loacc[0:64, hsl], rr[:])
                nc.sync.dma_start(ag_in2[half][:], oht[:])
                nc.gpsimd.collective_compute(
                    "AllGather", mybir.AluOpType.bypass,
                    replica_groups=groups,
                    ins=[ag_in2[half][:]], outs=[ag_out2[half][:]])
            # readback per half: rows 0:128 = kc6, 128:256 = kc7
            o_rb67 = []
            for half in range(2):
                tiles = []
                for j in range(2):
                    t = sb.tile([128, 512], BF16, name=f"orb67_{half}_{j}",
                                tag="orb", bufs=12)
                    nc.gpsimd.dma_start(
                        t[:], ag_out2[half][128 * j:128 * (j + 1), :])
                    tiles.append(t)
                o_rb67.append(tiles)
            # out-projection quadrants: kc 0..5 (heads 0-2, gathers already
            # landed) run during the final gathers; kc 6,7 finish after
            tail_jobs = [(n_chunks - 1, mc, n2, li0, lcw)
                         for mc in range(out_rows // 128)
                         for n2 in range(lcw // 512)]
            pps = []
            for j, (ihx, mc, n2, i0, cw) in enumerate(tail_jobs):
                pp = ps.tile([128, 512], F32, name=f"tpp{mc}_{n2}",
                             tag="sc" if j < 2 else "oacc", bufs=2)
                proj_mm(ihx, mc, n2, i0, pp, list(range(6)), True, False)
                pps.append(pp)
            for j, (ihx, mc, n2, i0, cw) in enumerate(tail_jobs):
                for kc in (6, 7):
                    nc.tensor.matmul(
                        pps[j][:],
                        wproj_sb[kc][:, 128 * mc:128 * (mc + 1)],
                        o_rb67[n2][kc - 6][:],
                        start=False, stop=(kc == 7))
                proj_evict(ihx, mc, n2, i0, pps[j])

    nc.compile()
    return nc


def shard_inputs(x, rope, w_qkv, b_qkv, w_proj, b_proj,
                 n_cores=N_CORES, group_size=4):
    """Per-core input maps. Host-side transposes/casts are part of sharding."""
    out_rows = C // group_size
    # fold the v-bias through the projection into an effective output bias
    b_v = b_qkv[2 * C:3 * C]
    b_eff = (b_proj + b_v @ w_proj.T).astype(np.float32)   # [C]

    in_maps = []
    for c in range(n_cores):
        b = (c // group_size) % B
        g = c % group_size
        heads = range(HL * g, HL * g + HL)

        xTb = np.ascontiguousarray(x[b].T).astype(BF)            # [C, N]

        cosT = rope[b].T[:D, :]                                   # [64, N]
        sinT = rope[b].T[D:, :]
        cos2 = np.vstack([cosT, cosT]).astype(BF)                 # [128, N]
        sgn = np.where(np.arange(128) % 2 == 0, -1.0, 1.0)[:, None]
        sin2s = (np.vstack([sinT, sinT]) * sgn).astype(BF)        # [128, N]

        # qk weight rows ordered [q_h0..q_h3, k_h0..k_h3]
        qk_rows = []
        bqk_rows = []
        for h in heads:
            qk_rows.append(w_qkv[D * h:D * (h + 1), :])           # q rows
            bqk_rows.append(b_qkv[D * h:D * (h + 1)])
        for h in heads:
            qk_rows.append(w_qkv[C + D * h:C + D * (h + 1), :])   # k rows
            bqk_rows.append(b_qkv[C + D * h:C + D * (h + 1)])
        wqk = np.vstack(qk_rows)                                  # [512, C]
        wqkT = np.ascontiguousarray(wqk.T).astype(BF)             # [C, 512]
        bqk_v = np.concatenate(bqk_rows).astype(np.float32)[:, None]

        h0 = HL * g
        wv = w_qkv[2 * C + D * h0:2 * C + D * h0 + CL, :]          # [256, C]
        wvT = np.ascontiguousarray(wv.T).astype(BF)                # [C, 256]

        # out-projection: this core owns output channels
        # [out_rows*g, out_rows*(g+1)).  The contraction rows are ordered
        # (head_local hl, rank r, d) to match the per-head AllGather layout
        # (gather hl concatenates rank blocks of 64 rows).
        osl = slice(out_rows * g, out_rows * (g + 1))
        wp_rows = []
        for hl in range(HL):
            for r in range(group_size):
                gh = HL * r + hl                   # global head of (r, hl)
                wp_rows.append(w_proj[osl, D * gh:D * (gh + 1)].T)  # [64,256]
        wprojT = np.ascontiguousarray(np.vstack(wp_rows)).astype(BF)  # [C,256]

        beff_shard = b_eff[osl].astype(np.float32)[:, None]

        in_maps.append({
            "xT": xTb, "cos2": cos2, "sin2s": sin2s,
            "wqkT": wqkT, "bqk": bqk_v, "wvT": wvT,
            "wprojT": wprojT, "beff": beff_shard,
        })
    return in_maps


def assemble(results, n_cores=N_CORES, group_size=4):
    out_rows = C // group_size
    out = np.empty((B, N, C), dtype=np.float32)
    for c in range(n_cores):
        b = (c // group_size) % B
        g = c % group_size
        outT_shard = results[c]["out"]                 # [out_rows, N] f32
        out[b, :, out_rows * g:out_rows * (g + 1)] = outT_shard.T
    return out


_NC_CACHE = {}


def _get_nc():
    if "nc" not in _NC_CACHE:
        _NC_CACHE["nc"] = build_kernel()
    return _NC_CACHE["nc"]


def _run(inputs, trace=False, tmpdir=None):
    nc = _get_nc()
    inputs = {k: np.asarray(v) for k, v in inputs.items()}
    in_maps = shard_inputs(**inputs)
    res = run_bass_kernel_spmd(nc, in_maps, core_ids=list(range(N_CORES)),
                               trace=trace, tmpdir=tmpdir)
    return assemble(res.results), res


def kernel(**inputs):
    out, _ = _run(inputs)
    return out
